# revision 10
# baseline (speedup 1.0000x reference)
"""Trainium2 Bass kernel for nn_ClusterLoss (vq_codebook).

reference:
    f = l2norm(features); c = l2norm(centers)
    sims = f @ c.T ; a = argmax(sims, -1)
    loss = mean(sum((f - centers[a])**2, -1))

Device algorithm (per core, data-parallel over N, 16384 rows each):
  per 128-row tile (PSUM tile [128,1024] f32):
    G  = beta * (f_bf16 @ c_hat_bf16.T)       # PE (2 matmuls, free=512)
    negbm = -max_k G                          # DVE reduce_max(negate) -> striped wides
    G += ln||c_k||  (rank-1 accumulate)       # PE (2 K=1 matmuls, after the max read)
    E  = exp(G + negbm);  S = sum_k E         # ACT, fused accum_out
       = sum_k ||c_k|| * e^{beta(G_k - m)}  ~= ||c_argmax||   (exact one-hot as beta->inf)
    ||f||^2 per row                           # GPSIMD scalar_tensor_tensor w/ sum accum
  per-row loss = 1 - 2*(m/beta)*S/||f|| + S^2 ; host sums losses (f64) across cores.

v2 changes vs the original baseline:
  - negbm accumulates into 8 striped wide tiles; ACT reads the bias column
    directly (no per-tile DVE bias copy, no tile-granular WAR hazard).
  - ||f||^2 fully on GPSIMD via scalar_tensor_tensor(accum_out) (no DVE
    norm reduce, no giant gpsimd multiply).
  - big input DMAs split into 8 chunks so compute starts early.

Identity: ||f_hat - c_a||^2 = 1 - 2*cos*||c_a|| + ||c_a||^2; argmax of cos-sim
is invariant to scaling by beta/||f||, so raw bf16 features feed the matmul.
"""
import os
import sys

sys.path.insert(0, "/opt/trn_rl_repo")

_OPT = os.environ.get("KOPT", "lag2,gbufs4,stripes,hostnorm,chunk8")

from contextlib import ExitStack

import numpy as np

import concourse.bass as bass
import concourse.bacc as bacc
import concourse.mybir as mybir
from concourse import tile
from concourse.bass_utils import run_bass_kernel_spmd

F32 = mybir.dt.float32
BF16 = mybir.dt.bfloat16
NP_BF16 = mybir.dt.np(mybir.dt.bfloat16)
AF = mybir.ActivationFunctionType
AX = mybir.AxisListType
ALU = mybir.AluOpType

N_CORES = 8
N_TOTAL = 131072
D = 128
K = 1024
ROWS_PER_CORE = N_TOTAL // N_CORES
BETA = 32768.0  # power of two: bf16(beta*c_hat) == beta*bf16(c_hat)

_nc_cache = {}


def build_nc(rows_per_core=ROWS_PER_CORE):
    return build_nc_rep(rows_per_core, rep=1)


def build_nc_rep(rows_per_core=ROWS_PER_CORE, rep=1):
    """Build + compile the per-core bass program (SPMD across 8 cores).

    rep>1 wraps the (idempotent) main loop in a hardware For_i loop --
    used for wall-clock HW timing with a constant-size NEFF.
    """
    if (rows_per_core, rep) in _nc_cache:
        return _nc_cache[(rows_per_core, rep)]

    R = rows_per_core
    T = R // 128  # number of 128-row tiles
    KT = K // 128  # center tiles (8)
    NCH = 8 if "chunk8" in _OPT else 1  # input DMA chunks
    TCH = T // NCH  # tiles per chunk

    nc = bacc.Bacc("TRN2", target_bir_lowering=False, debug=False, num_devices=N_CORES)

    HOSTNORM = "hostnorm" in _OPT
    ft = nc.dram_tensor("ft", [128, R], BF16, kind="ExternalInput").ap()  # f^T
    if HOSTNORM:
        nw_in = nc.dram_tensor("nw", [128, T], F32, kind="ExternalInput").ap()
    else:
        fn = nc.dram_tensor("fn", [R, 128], BF16, kind="ExternalInput").ap()
    cn = nc.dram_tensor("cn", [K, 128], F32, kind="ExternalInput").ap()  # centers
    ident = nc.dram_tensor("ident", [128, 128], BF16, kind="ExternalInput").ap()
    ones1 = nc.dram_tensor("ones1", [1, 128], BF16, kind="ExternalInput").ap()
    lossw = nc.dram_tensor("lossw", [128, T], F32, kind="ExternalOutput").ap()

    STRIPES = 8 if "stripes" in _OPT else 0
    LAG = 2 if "lag2" in _OPT else 1

    with tile.TileContext(nc) as tc, ExitStack() as ctx:
        const = ctx.enter_context(tc.tile_pool(name="const", bufs=1))
        setup = ctx.enter_context(tc.tile_pool(name="setup", bufs=2))
        setup_ps_cm = tc.tile_pool(name="setup_ps", bufs=1, space="PSUM")
        setup_ps = setup_ps_cm.__enter__()
        trash = ctx.enter_context(tc.tile_pool(name="trash", bufs=3))

        # ---------------- constants / big input loads ----------------
        id_sb = const.tile([128, 128], BF16)
        nc.sync.dma_start(id_sb[:], ident)
        on_sb = const.tile([1, 128], BF16)
        nc.sync.dma_start(on_sb[:], ones1)

        ct_sb = const.tile([128, KT * 128], F32)  # natural centers [kpart, (j d)]
        ct_v = ct_sb[:].rearrange("p (j d) -> p j d", d=128)
        nc.sync.dma_start(ct_v, cn.rearrange("(j p) d -> p j d", p=128))

        # chunked feature loads (compute on chunk 0 can start immediately)
        ft_c = []
        fn_c = []
        if not HOSTNORM:
            fn_r = fn.rearrange("(t p) d -> p t d", p=128)
        for i in range(NCH):
            ftt = const.tile([128, TCH * 128], BF16, name=f"ftc{i}")
            nc.sync.dma_start(ftt[:], ft[:, i * TCH * 128 : (i + 1) * TCH * 128])
            ft_c.append(ftt)
            if not HOSTNORM:
                fnt = const.tile([128, TCH * 128], BF16, name=f"fnc{i}")
                nc.sync.dma_start(
                    fnt[:].rearrange("p (t d) -> p t d", d=128),
                    fn_r[:, i * TCH : (i + 1) * TCH, :],
                )
                fn_c.append(fnt)

        def ft_blk(t):
            return ft_c[t // TCH][:, (t % TCH) * 128 : (t % TCH + 1) * 128]

        def fn_blk(t):
            return fn_c[t // TCH][:, (t % TCH) * 128 : (t % TCH + 1) * 128]

        if HOSTNORM:
            n2w_sb = const.tile([128, T], F32)
            nc.sync.dma_start(n2w_sb[:], nw_in)

        # ---------------- center setup ----------------
        # q[p, j] = ||c_{j*128+p}||^2
        qw = setup.tile([128, KT], F32)
        for j in range(KT):
            sq_t = trash.tile([128, 128], F32, tag="sq_t")
            nc.scalar.activation(
                sq_t[:], ct_v[:, j, :], AF.Square, accum_out=qw[:, j : j + 1]
            )
        # rinv = 1/||c||  (reciprocal then sqrt)
        qinv = setup.tile([128, KT], F32)
        nc.vector.reciprocal(qinv[:], qw[:])
        rinv = setup.tile([128, KT], F32)
        nc.scalar.activation(rinv[:], qinv[:], AF.Sqrt)
        # r = q * rinv = ||c||;  lnr = ln ||c||
        rr = setup.tile([128, KT], F32)
        nc.vector.tensor_mul(rr[:], qw[:], rinv[:])
        lnr = setup.tile([128, KT], F32)
        nc.scalar.activation(lnr[:], rr[:], AF.Ln)
        lnr_bf = setup.tile([128, KT], BF16)
        nc.vector.tensor_copy(lnr_bf[:], lnr[:])
        # scaled normalizer: beta / ||c||
        rinv_b = setup.tile([128, KT], F32)
        nc.vector.tensor_scalar_mul(rinv_b[:], rinv[:], float(BETA))

        # c_hat_scaled tiles (natural layout) then PE-transpose into chT [d, k]
        chT_sb = const.tile([128, K], BF16)
        for j in range(KT):
            ch_j = setup.tile([128, 128], BF16, tag="ch_j")
            nc.vector.tensor_scalar_mul(ch_j[:], ct_v[:, j, :], rinv_b[:, j : j + 1])
            chT_ps = setup_ps.tile([128, 128], BF16, tag="chT_ps")
            nc.tensor.transpose(chT_ps[:], ch_j[:], id_sb[:])
            nc.scalar.activation(
                chT_sb[:, j * 128 : (j + 1) * 128], chT_ps[:], AF.Copy
            )

        # lnr_row [1, K] at partition 0 via column-wise PE transposes
        lnr_row = const.tile([1, K], BF16)
        for j in range(KT):
            rt_ps = setup_ps.tile([1, 128], BF16, tag="rt_ps")
            nc.tensor.transpose(rt_ps[:], lnr_bf[:, j : j + 1], id_sb[:])
            nc.scalar.activation(
                lnr_row[:, j * 128 : (j + 1) * 128], rt_ps[:], AF.Copy
            )

        setup_ps_cm.__exit__(None, None, None)
        gpool = ctx.enter_context(tc.tile_pool(
            name="gpool", bufs=4 if "gbufs4" in _OPT else 3, space="PSUM"))
        epool = ctx.enter_context(tc.tile_pool(name="epool", bufs=3))
        bpool = ctx.enter_context(tc.tile_pool(name="bpool", bufs=6))

        # ---------------- wide accumulators ----------------
        sw = const.tile([128, T], F32)
        n2w = n2w_sb if HOSTNORM else const.tile([128, T], F32, name="n2w")
        if STRIPES:
            negbm_s = [
                const.tile([128, T // STRIPES], F32, name=f"nbm{j}", tag=f"nbm{j}")
                for j in range(STRIPES)
            ]

            def negbm_col(t):
                return negbm_s[t % STRIPES][:, t // STRIPES : t // STRIPES + 1]
        else:
            negbm_w0 = const.tile([128, T], F32)

        # ---------------- main loop (1-tile software pipeline) ----------------
        def emit_head(t):
            g_ps = gpool.tile([128, K], F32)
            lhs = ft_blk(t)
            nc.tensor.matmul(g_ps[:, 0:512], lhs, chT_sb[:, 0:512], start=True, stop=True)
            nc.tensor.matmul(g_ps[:, 512:1024], lhs, chT_sb[:, 512:1024], start=True, stop=True)
            if STRIPES:
                bias_ap = negbm_col(t)
                nc.vector.reduce_max(bias_ap, g_ps[:], axis=AX.X, negate=True)
            else:
                nc.vector.reduce_max(
                    negbm_w0[:, t : t + 1], g_ps[:], axis=AX.X, negate=True
                )
                bias_sm = bpool.tile([128, 1], F32, tag="bias")
                nc.vector.tensor_copy(bias_sm[:], negbm_w0[:, t : t + 1])
                bias_ap = bias_sm[:]
            return g_ps, bias_ap

        def emit_tail(t, g_ps, bias_ap):
            nc.tensor.matmul(g_ps[:, 0:512], on_sb[:], lnr_row[:, 0:512], start=False, stop=True, skip_group_check=True)
            nc.tensor.matmul(g_ps[:, 512:1024], on_sb[:], lnr_row[:, 512:1024], start=False, stop=True, skip_group_check=True)
            e_sb = epool.tile([128, K], BF16)
            nc.scalar.activation(
                e_sb[:], g_ps[:], AF.Exp,
                bias=bias_ap, scale=1.0,
                accum_out=sw[:, t : t + 1],
            )

        GN = 16  # norm-reduce batching group (devnorm only)

        def one_pass(_i=None):
            from collections import deque

            pend = deque()
            pend_norm = deque()

            def emit_norm_reduce():
                g0p, gnp, sq8p = pend_norm.popleft()
                nc.vector.reduce_sum(
                    n2w[:, g0p : g0p + gnp],
                    sq8p[:].rearrange("p (g d) -> p g d", d=128)[:, 0:gnp, :],
                    axis=AX.X,
                )

            for g0 in range(0, T, GN):
                gn = min(GN, T - g0)
                if not HOSTNORM:
                    assert GN == TCH, "devnorm requires chunk8 (GN == TCH)"
                    fnb = fn_c[g0 // TCH][:, 0 : gn * 128]
                    sq8 = trash.tile([128, GN * 128], F32, tag="sq8")
                    nc.gpsimd.tensor_mul(sq8[:, 0 : gn * 128], fnb, fnb)
                    pend_norm.append((g0, gn, sq8))
                for k, t in enumerate(range(g0, g0 + gn)):
                    g_ps, bias_ap = emit_head(t)
                    pend.append((t, g_ps, bias_ap))
                    if len(pend) > LAG:
                        emit_tail(*pend.popleft())
                    if not HOSTNORM and k == gn // 2 and len(pend_norm) > 1:
                        emit_norm_reduce()
            while pend:
                emit_tail(*pend.popleft())
            while pend_norm:
                emit_norm_reduce()

        if rep == 1:
            one_pass()
        else:
            with tc.For_i(0, rep) as _i:
                one_pass(_i)

        # ---------------- epilogue: per-row loss ----------------
        if STRIPES:
            negbm_w = setup.tile([128, T], F32)
            nv = negbm_w[:].rearrange("p (c j) -> p c j", j=STRIPES)
            for j in range(STRIPES):
                nc.vector.tensor_copy(nv[:, :, j], negbm_s[j][:])
        else:
            negbm_w = negbm_w0
        m_w = setup.tile([128, T], F32)
        nc.vector.tensor_scalar_mul(m_w[:], negbm_w[:], -1.0 / BETA)
        n2i = setup.tile([128, T], F32)
        nc.vector.reciprocal(n2i[:], n2w[:])
        invn = setup.tile([128, T], F32)
        nc.scalar.activation(invn[:], n2i[:], AF.Sqrt)
        a_w = setup.tile([128, T], F32)
        nc.vector.tensor_mul(a_w[:], m_w[:], invn[:])
        b_w = setup.tile([128, T], F32)
        nc.vector.tensor_mul(b_w[:], a_w[:], sw[:])
        b2_w = setup.tile([128, T], F32)
        nc.vector.tensor_scalar_mul(b2_w[:], b_w[:], -2.0)
        r2_w = setup.tile([128, T], F32)
        nc.vector.tensor_mul(r2_w[:], sw[:], sw[:])
        t3_w = setup.tile([128, T], F32)
        nc.vector.tensor_add(t3_w[:], r2_w[:], b2_w[:])
        lw = setup.tile([128, T], F32)
        nc.vector.tensor_scalar_add(lw[:], t3_w[:], 1.0)
        nc.sync.dma_start(lossw, lw[:])

    nc.compile()
    _nc_cache[(rows_per_core, rep)] = nc
    return nc


def make_in_maps(features, centers, rows_per_core=ROWS_PER_CORE, n_cores=N_CORES):
    f_bf = features.astype(NP_BF16)
    shards = f_bf.reshape(n_cores, rows_per_core, D)
    ident = np.eye(128, dtype=NP_BF16)
    ones1 = np.ones((1, 128), dtype=NP_BF16)
    cns = np.ascontiguousarray(centers.astype(np.float32))
    hostnorm = "hostnorm" in _OPT
    if hostnorm:
        # per-row ||f||^2 from the f32 features (exact), laid out [128, T]
        # to match the device's (t p) -> p t tiling
        n2 = np.square(features.astype(np.float32)).sum(axis=1)
        n2_shards = n2.reshape(n_cores, rows_per_core // 128, 128)
    in_maps = []
    for c in range(n_cores):
        s = shards[c]
        m = {
            "ft": np.ascontiguousarray(s.T),
            "cn": cns,
            "ident": ident,
            "ones1": ones1,
        }
        if hostnorm:
            m["nw"] = np.ascontiguousarray(n2_shards[c].T)
        else:
            m["fn"] = np.ascontiguousarray(s)
        in_maps.append(m)
    return in_maps


def kernel(features, centers):
    features = np.asarray(features)
    centers = np.asarray(centers)
    nc = build_nc(ROWS_PER_CORE)
    in_maps = make_in_maps(features, centers)
    res = run_bass_kernel_spmd(nc, in_maps, core_ids=list(range(N_CORES)))
    total = 0.0
    for c in range(N_CORES):
        total += res.results[c]["lossw"].astype(np.float64).sum()
    return np.float32(total / (ROWS_PER_CORE * N_CORES))


# revision 11
# speedup vs baseline: 1.1843x; 1.1843x over previous
"""Trainium2 Bass kernel for nn_ClusterLoss (vq_codebook).

reference:
    f = l2norm(features); c = l2norm(centers)
    sims = f @ c.T ; a = argmax(sims, -1)
    loss = mean(sum((f - centers[a])**2, -1))

Device algorithm (per core, data-parallel over N, 16384 rows each):
  per 128-row tile (PSUM tile [128,1024] f32):
    G  = beta * (f_bf16 @ c_hat_bf16.T)       # PE (2 matmuls, free=512)
    negbm = -max_k G                          # DVE reduce_max(negate) -> striped wides
    G += ln||c_k||  (rank-1 accumulate)       # PE (2 K=1 matmuls, after the max read)
    E  = exp(G + negbm);  S = sum_k E         # ACT, fused accum_out
       = sum_k ||c_k|| * e^{beta(G_k - m)}  ~= ||c_argmax||   (exact one-hot as beta->inf)
    ||f||^2 per row                           # GPSIMD scalar_tensor_tensor w/ sum accum
  per-row loss = 1 - 2*(m/beta)*S/||f|| + S^2 ; host sums losses (f64) across cores.

v2 changes vs the original baseline:
  - negbm accumulates into 8 striped wide tiles; ACT reads the bias column
    directly (no per-tile DVE bias copy, no tile-granular WAR hazard).
  - ||f||^2 fully on GPSIMD via scalar_tensor_tensor(accum_out) (no DVE
    norm reduce, no giant gpsimd multiply).
  - big input DMAs split into 8 chunks so compute starts early.

Identity: ||f_hat - c_a||^2 = 1 - 2*cos*||c_a|| + ||c_a||^2; argmax of cos-sim
is invariant to scaling by beta/||f||, so raw bf16 features feed the matmul.
"""
import os
import sys

sys.path.insert(0, "/opt/trn_rl_repo")

_OPT = os.environ.get("KOPT", "lag2,gbufs4,stripes,hostnorm,chunk8")

from contextlib import ExitStack

import numpy as np

import concourse.bass as bass
import concourse.bacc as bacc
import concourse.mybir as mybir
from concourse import tile
from concourse.bass_utils import run_bass_kernel_spmd

F32 = mybir.dt.float32
BF16 = mybir.dt.bfloat16
NP_BF16 = mybir.dt.np(mybir.dt.bfloat16)
AF = mybir.ActivationFunctionType
AX = mybir.AxisListType
ALU = mybir.AluOpType

N_CORES = 8
N_TOTAL = 131072
D = 128
K = 1024
ROWS_PER_CORE = N_TOTAL // N_CORES
BETA = 32768.0  # power of two: bf16(beta*c_hat) == beta*bf16(c_hat)

_nc_cache = {}


def build_nc(rows_per_core=ROWS_PER_CORE):
    return build_nc_rep(rows_per_core, rep=1)


def build_nc_rep(rows_per_core=ROWS_PER_CORE, rep=1):
    """Build + compile the per-core bass program (SPMD across 8 cores).

    rep>1 wraps the (idempotent) main loop in a hardware For_i loop --
    used for wall-clock HW timing with a constant-size NEFF.
    """
    if (rows_per_core, rep) in _nc_cache:
        return _nc_cache[(rows_per_core, rep)]

    R = rows_per_core
    T = R // 128  # number of 128-row tiles
    KT = K // 128  # center tiles (8)
    NCH = 8 if "chunk8" in _OPT else 1  # input DMA chunks
    TCH = T // NCH  # tiles per chunk

    nc = bacc.Bacc("TRN2", target_bir_lowering=False, debug=False, num_devices=N_CORES)

    HOSTNORM = "hostnorm" in _OPT
    ft = nc.dram_tensor("ft", [128, R], BF16, kind="ExternalInput").ap()  # f^T
    if HOSTNORM:
        nw_in = nc.dram_tensor("nw", [128, T], F32, kind="ExternalInput").ap()
    else:
        fn = nc.dram_tensor("fn", [R, 128], BF16, kind="ExternalInput").ap()
    cn = nc.dram_tensor("cn", [K, 128], F32, kind="ExternalInput").ap()  # centers
    ident = nc.dram_tensor("ident", [128, 128], BF16, kind="ExternalInput").ap()
    ones1 = nc.dram_tensor("ones1", [1, 128], BF16, kind="ExternalInput").ap()
    lossw = nc.dram_tensor("lossw", [128, T], F32, kind="ExternalOutput").ap()

    STRIPES = 8 if "stripes" in _OPT else 0
    LAG = 2 if "lag2" in _OPT else 1

    with tile.TileContext(nc) as tc, ExitStack() as ctx:
        const = ctx.enter_context(tc.tile_pool(name="const", bufs=1))
        setup = ctx.enter_context(tc.tile_pool(name="setup", bufs=2))
        setup_ps_cm = tc.tile_pool(name="setup_ps", bufs=1, space="PSUM")
        setup_ps = setup_ps_cm.__enter__()
        trash = ctx.enter_context(tc.tile_pool(name="trash", bufs=3))

        # ---------------- constants / big input loads ----------------
        id_sb = const.tile([128, 128], BF16)
        nc.sync.dma_start(id_sb[:], ident)
        on_sb = const.tile([1, 128], BF16)
        nc.sync.dma_start(on_sb[:], ones1)

        ct_sb = const.tile([128, KT * 128], F32)  # natural centers [kpart, (j d)]
        ct_v = ct_sb[:].rearrange("p (j d) -> p j d", d=128)
        nc.sync.dma_start(ct_v, cn.rearrange("(j p) d -> p j d", p=128))

        # chunked feature loads (compute on chunk 0 can start immediately)
        ft_c = []
        fn_c = []
        if not HOSTNORM:
            fn_r = fn.rearrange("(t p) d -> p t d", p=128)
        for i in range(NCH):
            ftt = const.tile([128, TCH * 128], BF16, name=f"ftc{i}")
            nc.sync.dma_start(ftt[:], ft[:, i * TCH * 128 : (i + 1) * TCH * 128])
            ft_c.append(ftt)
            if not HOSTNORM:
                fnt = const.tile([128, TCH * 128], BF16, name=f"fnc{i}")
                nc.sync.dma_start(
                    fnt[:].rearrange("p (t d) -> p t d", d=128),
                    fn_r[:, i * TCH : (i + 1) * TCH, :],
                )
                fn_c.append(fnt)

        def ft_blk(t):
            return ft_c[t // TCH][:, (t % TCH) * 128 : (t % TCH + 1) * 128]

        def fn_blk(t):
            return fn_c[t // TCH][:, (t % TCH) * 128 : (t % TCH + 1) * 128]

        if HOSTNORM:
            n2w_sb = const.tile([128, T], F32)
            nc.sync.dma_start(n2w_sb[:], nw_in)

        # ---------------- center setup ----------------
        # q[p, j] = ||c_{j*128+p}||^2
        qw = setup.tile([128, KT], F32)
        for j in range(KT):
            sq_t = trash.tile([128, 128], F32, tag="sq_t")
            nc.scalar.activation(
                sq_t[:], ct_v[:, j, :], AF.Square, accum_out=qw[:, j : j + 1]
            )
        # rinv = 1/||c||  (reciprocal then sqrt)
        qinv = setup.tile([128, KT], F32)
        nc.vector.reciprocal(qinv[:], qw[:])
        rinv = setup.tile([128, KT], F32)
        nc.scalar.activation(rinv[:], qinv[:], AF.Sqrt)
        # r = q * rinv = ||c||;  lnr = ln ||c||
        rr = setup.tile([128, KT], F32)
        nc.vector.tensor_mul(rr[:], qw[:], rinv[:])
        lnr = setup.tile([128, KT], F32)
        nc.scalar.activation(lnr[:], rr[:], AF.Ln)
        lnr_bf = setup.tile([128, KT], BF16)
        nc.vector.tensor_copy(lnr_bf[:], lnr[:])
        # scaled normalizer: beta / ||c||
        rinv_b = setup.tile([128, KT], F32)
        nc.vector.tensor_scalar_mul(rinv_b[:], rinv[:], float(BETA))

        # c_hat_scaled tiles (natural layout) then PE-transpose into chT [d, k]
        chT_sb = const.tile([128, K], BF16)
        for j in range(KT):
            ch_j = setup.tile([128, 128], BF16, tag="ch_j")
            nc.vector.tensor_scalar_mul(ch_j[:], ct_v[:, j, :], rinv_b[:, j : j + 1])
            chT_ps = setup_ps.tile([128, 128], BF16, tag="chT_ps")
            nc.tensor.transpose(chT_ps[:], ch_j[:], id_sb[:])
            nc.scalar.activation(
                chT_sb[:, j * 128 : (j + 1) * 128], chT_ps[:], AF.Copy
            )

        # lnr_row [1, K] at partition 0 via column-wise PE transposes
        lnr_row = const.tile([1, K], BF16)
        for j in range(KT):
            rt_ps = setup_ps.tile([1, 128], BF16, tag="rt_ps")
            nc.tensor.transpose(rt_ps[:], lnr_bf[:, j : j + 1], id_sb[:])
            nc.scalar.activation(
                lnr_row[:, j * 128 : (j + 1) * 128], rt_ps[:], AF.Copy
            )

        setup_ps_cm.__exit__(None, None, None)
        gpool = ctx.enter_context(tc.tile_pool(
            name="gpool", bufs=4 if "gbufs4" in _OPT else 3, space="PSUM"))
        epool = ctx.enter_context(tc.tile_pool(name="epool", bufs=3))
        bpool = ctx.enter_context(tc.tile_pool(name="bpool", bufs=6))

        # ---------------- wide accumulators ----------------
        sw = const.tile([128, T], F32)
        n2w = n2w_sb if HOSTNORM else const.tile([128, T], F32, name="n2w")
        if STRIPES:
            negbm_s = [
                const.tile([128, T // STRIPES], F32, name=f"nbm{j}", tag=f"nbm{j}")
                for j in range(STRIPES)
            ]

            def negbm_col(t):
                return negbm_s[t % STRIPES][:, t // STRIPES : t // STRIPES + 1]
        else:
            negbm_w0 = const.tile([128, T], F32)

        # ---------------- main loop (1-tile software pipeline) ----------------
        def emit_head(t):
            g_ps = gpool.tile([128, K], F32)
            lhs = ft_blk(t)
            nc.tensor.matmul(g_ps[:, 0:512], lhs, chT_sb[:, 0:512], start=True, stop=True)
            nc.tensor.matmul(g_ps[:, 512:1024], lhs, chT_sb[:, 512:1024], start=True, stop=True)
            if STRIPES:
                bias_ap = negbm_col(t)
                nc.vector.reduce_max(bias_ap, g_ps[:], axis=AX.X, negate=True)
            else:
                nc.vector.reduce_max(
                    negbm_w0[:, t : t + 1], g_ps[:], axis=AX.X, negate=True
                )
                bias_sm = bpool.tile([128, 1], F32, tag="bias")
                nc.vector.tensor_copy(bias_sm[:], negbm_w0[:, t : t + 1])
                bias_ap = bias_sm[:]
            return g_ps, bias_ap

        def emit_tail(t, g_ps, bias_ap):
            nc.tensor.matmul(g_ps[:, 0:512], on_sb[:], lnr_row[:, 0:512], start=False, stop=True, skip_group_check=True)
            nc.tensor.matmul(g_ps[:, 512:1024], on_sb[:], lnr_row[:, 512:1024], start=False, stop=True, skip_group_check=True)
            e_sb = epool.tile([128, K], BF16)
            nc.scalar.activation(
                e_sb[:], g_ps[:], AF.Exp,
                bias=bias_ap, scale=1.0,
                accum_out=sw[:, t : t + 1],
            )

        GN = 16  # norm-reduce batching group (devnorm only)

        def one_pass(_i=None):
            from collections import deque

            pend = deque()
            pend_norm = deque()

            def emit_norm_reduce():
                g0p, gnp, sq8p = pend_norm.popleft()
                nc.vector.reduce_sum(
                    n2w[:, g0p : g0p + gnp],
                    sq8p[:].rearrange("p (g d) -> p g d", d=128)[:, 0:gnp, :],
                    axis=AX.X,
                )

            for g0 in range(0, T, GN):
                gn = min(GN, T - g0)
                if not HOSTNORM:
                    assert GN == TCH, "devnorm requires chunk8 (GN == TCH)"
                    fnb = fn_c[g0 // TCH][:, 0 : gn * 128]
                    sq8 = trash.tile([128, GN * 128], F32, tag="sq8")
                    nc.gpsimd.tensor_mul(sq8[:, 0 : gn * 128], fnb, fnb)
                    pend_norm.append((g0, gn, sq8))
                for k, t in enumerate(range(g0, g0 + gn)):
                    g_ps, bias_ap = emit_head(t)
                    pend.append((t, g_ps, bias_ap))
                    if len(pend) > LAG:
                        emit_tail(*pend.popleft())
                    if not HOSTNORM and k == gn // 2 and len(pend_norm) > 1:
                        emit_norm_reduce()
            while pend:
                emit_tail(*pend.popleft())
            while pend_norm:
                emit_norm_reduce()

        if rep == 1:
            one_pass()
        elif rep < 0:  # python-unrolled (for TimelineSim steady-state checks)
            for _ in range(-rep):
                one_pass()
        else:
            with tc.For_i(0, rep) as _i:
                one_pass(_i)

        # ---------------- epilogue: per-row loss ----------------
        if STRIPES:
            negbm_w = setup.tile([128, T], F32)
            nv = negbm_w[:].rearrange("p (c j) -> p c j", j=STRIPES)
            for j in range(STRIPES):
                nc.vector.tensor_copy(nv[:, :, j], negbm_s[j][:])
        else:
            negbm_w = negbm_w0
        m_w = setup.tile([128, T], F32)
        nc.vector.tensor_scalar_mul(m_w[:], negbm_w[:], -1.0 / BETA)
        n2i = setup.tile([128, T], F32)
        nc.vector.reciprocal(n2i[:], n2w[:])
        invn = setup.tile([128, T], F32)
        nc.scalar.activation(invn[:], n2i[:], AF.Sqrt)
        a_w = setup.tile([128, T], F32)
        nc.vector.tensor_mul(a_w[:], m_w[:], invn[:])
        b_w = setup.tile([128, T], F32)
        nc.vector.tensor_mul(b_w[:], a_w[:], sw[:])
        b2_w = setup.tile([128, T], F32)
        nc.vector.tensor_scalar_mul(b2_w[:], b_w[:], -2.0)
        r2_w = setup.tile([128, T], F32)
        nc.vector.tensor_mul(r2_w[:], sw[:], sw[:])
        t3_w = setup.tile([128, T], F32)
        nc.vector.tensor_add(t3_w[:], r2_w[:], b2_w[:])
        lw = setup.tile([128, T], F32)
        nc.vector.tensor_scalar_add(lw[:], t3_w[:], 1.0)
        nc.sync.dma_start(lossw, lw[:])

    nc.compile()
    _nc_cache[(rows_per_core, rep)] = nc
    return nc


def make_in_maps(features, centers, rows_per_core=ROWS_PER_CORE, n_cores=N_CORES):
    f_bf = features.astype(NP_BF16)
    shards = f_bf.reshape(n_cores, rows_per_core, D)
    ident = np.eye(128, dtype=NP_BF16)
    ones1 = np.ones((1, 128), dtype=NP_BF16)
    cns = np.ascontiguousarray(centers.astype(np.float32))
    hostnorm = "hostnorm" in _OPT
    if hostnorm:
        # per-row ||f||^2 from the f32 features (exact), laid out [128, T]
        # to match the device's (t p) -> p t tiling
        n2 = np.square(features.astype(np.float32)).sum(axis=1)
        n2_shards = n2.reshape(n_cores, rows_per_core // 128, 128)
    in_maps = []
    for c in range(n_cores):
        s = shards[c]
        m = {
            "ft": np.ascontiguousarray(s.T),
            "cn": cns,
            "ident": ident,
            "ones1": ones1,
        }
        if hostnorm:
            m["nw"] = np.ascontiguousarray(n2_shards[c].T)
        else:
            m["fn"] = np.ascontiguousarray(s)
        in_maps.append(m)
    return in_maps


def kernel(features, centers):
    features = np.asarray(features)
    centers = np.asarray(centers)
    nc = build_nc(ROWS_PER_CORE)
    in_maps = make_in_maps(features, centers)
    res = run_bass_kernel_spmd(nc, in_maps, core_ids=list(range(N_CORES)))
    total = 0.0
    for c in range(N_CORES):
        total += res.results[c]["lossw"].astype(np.float64).sum()
    return np.float32(total / (ROWS_PER_CORE * N_CORES))


# revision 12
# speedup vs baseline: 1.9342x; 1.6332x over previous
"""Trainium2 Bass kernel for nn_ClusterLoss (vq_codebook).

reference:
    f = l2norm(features); c = l2norm(centers)
    sims = f @ c.T ; a = argmax(sims, -1)
    loss = mean(sum((f - centers[a])**2, -1))

Device algorithm (per core, data-parallel over N, 16384 rows each),
3-hop pipeline PE -> DVE -> ACT per 128-row tile:
  PE : H = lnr_bcast + beta*(f_bf16 @ c_hat_bf16.T)   (PSUM, one accum group:
       K=128 broadcast matmul pair preloads ln||c_k||, then the 2 main
       matmuls accumulate G; K=1 rank-1 matmuls are ~3x slower on HW, so the
       lnr row is injected via (ones/128) @ lnr_bcast instead)
  DVE: custom op NEG_SUB_MIN_REDUCE (see dve_ext):
       negbm = min_k(lnr_k - H_k) = -max_k G  (the exp bias), out = trash
  ACT: E = exp(H + negbm); S = sum_k E  (fused accum_out)
       = sum_k ||c_k||*e^{beta(cos_k - max cos)} ~= ||c_argmax||
  per-row loss = 1 - 2*(m/beta)*S/||f|| + S^2; ||f||^2 comes in as a
  host-computed input (exact, from the f32 features); host sums losses (f64).

Identity: ||f_hat - c_a||^2 = 1 - 2*cos*||c_a|| + ||c_a||^2; argmax of cos-sim
is invariant to scaling by beta/||f||, so raw bf16 features feed the matmul.
"""
import os
import sys

sys.path.insert(0, "/opt/trn_rl_repo")
sys.path.insert(0, "/root/problem")

_OPT = os.environ.get("KOPT", "lag2,gbufs4,stripes,hostnorm,chunk8,cdve")

from contextlib import ExitStack

import numpy as np

import concourse.bass as bass
import concourse.bacc as bacc
import concourse.mybir as mybir
from concourse import tile
from concourse.bass_utils import run_bass_kernel_spmd

import dve_ext

F32 = mybir.dt.float32
BF16 = mybir.dt.bfloat16
NP_BF16 = mybir.dt.np(mybir.dt.bfloat16)
AF = mybir.ActivationFunctionType
AX = mybir.AxisListType

N_CORES = 8
N_TOTAL = 131072
D = 128
K = 1024
ROWS_PER_CORE = N_TOTAL // N_CORES
BETA = 32768.0  # power of two: bf16(beta*c_hat) == beta*bf16(c_hat)

_nc_cache = {}


def build_nc(rows_per_core=ROWS_PER_CORE):
    return build_nc_rep(rows_per_core, rep=1)


def build_nc_rep(rows_per_core=ROWS_PER_CORE, rep=1):
    """Build + compile the per-core bass program (SPMD across 8 cores).

    rep>1 wraps the (idempotent) main loop in a hardware For_i loop --
    used for wall-clock HW timing with a constant-size NEFF. rep<0 unrolls
    |rep| passes in python (for TimelineSim steady-state checks).
    """
    if (rows_per_core, rep) in _nc_cache:
        return _nc_cache[(rows_per_core, rep)]

    R = rows_per_core
    T = R // 128  # number of 128-row tiles
    KT = K // 128  # center tiles (8)
    NCH = 8 if "chunk8" in _OPT else 1  # input DMA chunks
    TCH = T // NCH  # tiles per chunk
    CDVE = "cdve" in _OPT

    nc = bacc.Bacc("TRN2", target_bir_lowering=False, debug=False, num_devices=N_CORES)

    ft = nc.dram_tensor("ft", [128, R], BF16, kind="ExternalInput").ap()  # f^T
    nw_in = nc.dram_tensor("nw", [128, T], F32, kind="ExternalInput").ap()
    cn = nc.dram_tensor("cn", [K, 128], F32, kind="ExternalInput").ap()  # centers
    ident = nc.dram_tensor("ident", [128, 128], BF16, kind="ExternalInput").ap()
    m128 = nc.dram_tensor("m128", [128, 128], BF16, kind="ExternalInput").ap()
    ones1 = nc.dram_tensor("ones1", [1, 128], BF16, kind="ExternalInput").ap()
    lossw = nc.dram_tensor("lossw", [128, T], F32, kind="ExternalOutput").ap()

    STRIPES = 8 if "stripes" in _OPT else 0
    LAG = 2 if "lag2" in _OPT else 1

    with tile.TileContext(nc) as tc, ExitStack() as ctx:
        const = ctx.enter_context(tc.tile_pool(name="const", bufs=1))
        setup = ctx.enter_context(tc.tile_pool(name="setup", bufs=2))
        setup_ps_cm = tc.tile_pool(name="setup_ps", bufs=1, space="PSUM")
        setup_ps = setup_ps_cm.__enter__()
        trash = ctx.enter_context(tc.tile_pool(name="trash", bufs=3))

        # ---------------- constants / big input loads ----------------
        id_sb = const.tile([128, 128], BF16)
        nc.sync.dma_start(id_sb[:], ident)
        m128_sb = const.tile([128, 128], BF16)
        nc.sync.dma_start(m128_sb[:], m128)
        on_sb = const.tile([1, 128], BF16)
        nc.sync.dma_start(on_sb[:], ones1)

        ct_sb = const.tile([128, KT * 128], F32)  # natural centers [kpart, (j d)]
        ct_v = ct_sb[:].rearrange("p (j d) -> p j d", d=128)
        nc.sync.dma_start(ct_v, cn.rearrange("(j p) d -> p j d", p=128))

        n2w = const.tile([128, T], F32)
        nc.sync.dma_start(n2w[:], nw_in)

        # chunked feature loads (compute on chunk 0 can start immediately)
        ft_c = []
        for i in range(NCH):
            ftt = const.tile([128, TCH * 128], BF16, name=f"ftc{i}")
            nc.sync.dma_start(ftt[:], ft[:, i * TCH * 128 : (i + 1) * TCH * 128])
            ft_c.append(ftt)

        def ft_blk(t):
            return ft_c[t // TCH][:, (t % TCH) * 128 : (t % TCH + 1) * 128]

        # ---------------- center setup ----------------
        # q[p, j] = ||c_{j*128+p}||^2
        qw = setup.tile([128, KT], F32)
        for j in range(KT):
            sq_t = trash.tile([128, 128], F32, tag="sq_t")
            nc.scalar.activation(
                sq_t[:], ct_v[:, j, :], AF.Square, accum_out=qw[:, j : j + 1]
            )
        # rinv = 1/||c||  (reciprocal then sqrt)
        qinv = setup.tile([128, KT], F32)
        nc.vector.reciprocal(qinv[:], qw[:])
        rinv = setup.tile([128, KT], F32)
        nc.scalar.activation(rinv[:], qinv[:], AF.Sqrt)
        # r = q * rinv = ||c||;  lnr = ln ||c||
        rr = setup.tile([128, KT], F32)
        nc.vector.tensor_mul(rr[:], qw[:], rinv[:])
        lnr = setup.tile([128, KT], F32)
        nc.scalar.activation(lnr[:], rr[:], AF.Ln)
        lnr_bf = setup.tile([128, KT], BF16)
        nc.vector.tensor_copy(lnr_bf[:], lnr[:])
        # scaled normalizer: beta / ||c||
        rinv_b = setup.tile([128, KT], F32)
        nc.vector.tensor_scalar_mul(rinv_b[:], rinv[:], float(BETA))

        # c_hat_scaled tiles (natural layout) then PE-transpose into chT [d, k]
        chT_sb = const.tile([128, K], BF16)
        for j in range(KT):
            ch_j = setup.tile([128, 128], BF16, tag="ch_j")
            nc.vector.tensor_scalar_mul(ch_j[:], ct_v[:, j, :], rinv_b[:, j : j + 1])
            chT_ps = setup_ps.tile([128, 128], BF16, tag="chT_ps")
            nc.tensor.transpose(chT_ps[:], ch_j[:], id_sb[:])
            nc.scalar.activation(
                chT_sb[:, j * 128 : (j + 1) * 128], chT_ps[:], AF.Copy
            )

        # lnr_row [1, K] at partition 0 via column-wise PE transposes
        lnr_row = setup.tile([1, K], BF16)
        for j in range(KT):
            rt_ps = setup_ps.tile([1, 128], BF16, tag="rt_ps")
            nc.tensor.transpose(rt_ps[:], lnr_bf[:, j : j + 1], id_sb[:])
            nc.scalar.activation(
                lnr_row[:, j * 128 : (j + 1) * 128], rt_ps[:], AF.Copy
            )

        # LNR_bcast [128, K] bf16: lnr_row broadcast down all partitions
        # (K=1 matmul pair, setup-only), copied out through ACT.
        lnr_bc = const.tile([128, K], BF16)
        for h in range(2):
            bc_ps = setup_ps.tile([128, 512], F32, tag="bc_ps")
            nc.tensor.matmul(
                bc_ps[:], on_sb[:], lnr_row[:, h * 512 : (h + 1) * 512],
                start=True, stop=True,
            )
            nc.scalar.activation(
                lnr_bc[:, h * 512 : (h + 1) * 512], bc_ps[:], AF.Copy
            )

        setup_ps_cm.__exit__(None, None, None)
        gpool = ctx.enter_context(tc.tile_pool(
            name="gpool", bufs=4 if "gbufs4" in _OPT else 3, space="PSUM"))
        epool = ctx.enter_context(tc.tile_pool(name="epool", bufs=3))
        dpool = ctx.enter_context(tc.tile_pool(name="dpool", bufs=3))

        # ---------------- wide accumulators ----------------
        sw = const.tile([128, T], F32)
        if STRIPES:
            negbm_s = [
                const.tile([128, T // STRIPES], F32, name=f"nbm{j}", tag=f"nbm{j}")
                for j in range(STRIPES)
            ]

            def negbm_col(t):
                return negbm_s[t % STRIPES][:, t // STRIPES : t // STRIPES + 1]
        else:
            negbm_w0 = const.tile([128, T], F32)

            def negbm_col(t):
                return negbm_w0[:, t : t + 1]

        # ---------------- main loop (software pipeline) ----------------
        def emit_head(t):
            g_ps = gpool.tile([128, K], F32)
            lhs = ft_blk(t)
            if CDVE:
                # preload lnr (K=128 broadcast matmul), then accumulate G
                nc.tensor.matmul(g_ps[:, 0:512], m128_sb[:], lnr_bc[:, 0:512], start=True, stop=False, skip_group_check=True)
                nc.tensor.matmul(g_ps[:, 512:1024], m128_sb[:], lnr_bc[:, 512:1024], start=True, stop=False, skip_group_check=True)
                nc.tensor.matmul(g_ps[:, 0:512], lhs, chT_sb[:, 0:512], start=False, stop=True, skip_group_check=True)
                nc.tensor.matmul(g_ps[:, 512:1024], lhs, chT_sb[:, 512:1024], start=False, stop=True, skip_group_check=True)
                bias_ap = negbm_col(t)
                dout = dpool.tile([128, K], BF16, tag="dout")
                dve_ext.neg_sub_min_reduce(nc, dout[:], g_ps[:], lnr_bc[:], bias_ap)
            else:
                nc.tensor.matmul(g_ps[:, 0:512], lhs, chT_sb[:, 0:512], start=True, stop=True)
                nc.tensor.matmul(g_ps[:, 512:1024], lhs, chT_sb[:, 512:1024], start=True, stop=True)
                bias_ap = negbm_col(t)
                nc.vector.reduce_max(bias_ap, g_ps[:], axis=AX.X, negate=True)
            return g_ps, bias_ap

        def emit_tail(t, g_ps, bias_ap):
            if not CDVE:
                # lnr add via K=128 broadcast matmul (K=1 rank-1s are ~3x slower)
                nc.tensor.matmul(g_ps[:, 0:512], m128_sb[:], lnr_bc[:, 0:512], start=False, stop=True, skip_group_check=True)
                nc.tensor.matmul(g_ps[:, 512:1024], m128_sb[:], lnr_bc[:, 512:1024], start=False, stop=True, skip_group_check=True)
            e_sb = epool.tile([128, K], BF16)
            nc.scalar.activation(
                e_sb[:], g_ps[:], AF.Exp,
                bias=bias_ap, scale=1.0,
                accum_out=sw[:, t : t + 1],
            )

        def one_pass(_i=None):
            from collections import deque

            pend = deque()
            for t in range(T):
                g_ps, bias_ap = emit_head(t)
                pend.append((t, g_ps, bias_ap))
                if len(pend) > LAG:
                    emit_tail(*pend.popleft())
            while pend:
                emit_tail(*pend.popleft())

        if rep == 1:
            one_pass()
        elif rep < 0:  # python-unrolled (for TimelineSim steady-state checks)
            for _ in range(-rep):
                one_pass()
        else:
            with tc.For_i(0, rep) as _i:
                one_pass(_i)

        # ---------------- epilogue: per-row loss ----------------
        if STRIPES:
            negbm_w = setup.tile([128, T], F32)
            nv = negbm_w[:].rearrange("p (c j) -> p c j", j=STRIPES)
            for j in range(STRIPES):
                nc.vector.tensor_copy(nv[:, :, j], negbm_s[j][:])
        else:
            negbm_w = negbm_w0
        m_w = setup.tile([128, T], F32)
        nc.vector.tensor_scalar_mul(m_w[:], negbm_w[:], -1.0 / BETA)
        n2i = setup.tile([128, T], F32)
        nc.vector.reciprocal(n2i[:], n2w[:])
        invn = setup.tile([128, T], F32)
        nc.scalar.activation(invn[:], n2i[:], AF.Sqrt)
        a_w = setup.tile([128, T], F32)
        nc.vector.tensor_mul(a_w[:], m_w[:], invn[:])
        b_w = setup.tile([128, T], F32)
        nc.vector.tensor_mul(b_w[:], a_w[:], sw[:])
        b2_w = setup.tile([128, T], F32)
        nc.vector.tensor_scalar_mul(b2_w[:], b_w[:], -2.0)
        r2_w = setup.tile([128, T], F32)
        nc.vector.tensor_mul(r2_w[:], sw[:], sw[:])
        t3_w = setup.tile([128, T], F32)
        nc.vector.tensor_add(t3_w[:], r2_w[:], b2_w[:])
        lw = setup.tile([128, T], F32)
        nc.vector.tensor_scalar_add(lw[:], t3_w[:], 1.0)
        nc.sync.dma_start(lossw, lw[:])

    nc.compile()
    _nc_cache[(rows_per_core, rep)] = nc
    return nc


def make_in_maps(features, centers, rows_per_core=ROWS_PER_CORE, n_cores=N_CORES):
    f_bf = features.astype(NP_BF16)
    shards = f_bf.reshape(n_cores, rows_per_core, D)
    ident = np.eye(128, dtype=NP_BF16)
    m128 = np.full((128, 128), 1.0 / 128.0, dtype=NP_BF16)
    ones1 = np.ones((1, 128), dtype=NP_BF16)
    cns = np.ascontiguousarray(centers.astype(np.float32))
    # per-row ||f||^2 from the f32 features (exact), laid out [128, T]
    n2 = np.square(features.astype(np.float32)).sum(axis=1)
    n2_shards = n2.reshape(n_cores, rows_per_core // 128, 128)
    in_maps = []
    for c in range(n_cores):
        s = shards[c]
        in_maps.append(
            {
                "ft": np.ascontiguousarray(s.T),
                "nw": np.ascontiguousarray(n2_shards[c].T),
                "cn": cns,
                "ident": ident,
                "m128": m128,
                "ones1": ones1,
            }
        )
    return in_maps


def kernel(features, centers):
    features = np.asarray(features)
    centers = np.asarray(centers)
    nc = build_nc(ROWS_PER_CORE)
    in_maps = make_in_maps(features, centers)
    res = run_bass_kernel_spmd(nc, in_maps, core_ids=list(range(N_CORES)))
    total = 0.0
    for c in range(N_CORES):
        total += res.results[c]["lossw"].astype(np.float64).sum()
    return np.float32(total / (ROWS_PER_CORE * N_CORES))
